# revision 1
# baseline (speedup 1.0000x reference)
"""MoE FeedForward (top-2 of 8 experts + shared expert + LayerNorm) on 8 TRN2 NeuronCores.

Strategy: data-parallel over tokens (2048 tokens/core, zero collectives).
Per core: fp32 gate matmul -> top-2 routing masks/weights via vector ops ->
per-expert token lists via prefix-sum + sparse_gather compaction ->
dma_gather (transposed, bf16) dispatch -> bf16 expert FFN (fp32 PSUM) into
a slot-indexed Ycat buffer -> ap_gather combine with softmax weights ->
shared expert + residual -> LayerNorm (partition-dim reduce via PE matmul).

kernel(**inputs) takes full-size numpy inputs, returns [16384, 1024] fp32.
"""
import os
import numpy as np
import ml_dtypes
_PH = int(os.environ.get("KPH", "5"))

import concourse.bacc as bacc
import concourse.mybir as mybir
import concourse.tile as tile

B = 16384
D = 1024
E = 8
F = 2048
NCORE = 8
N = B // NCORE
C = 640                 # per-expert capacity per core (max seed count 559)
CW = C // 16
NE = E * C
DC = D // 128
FC = F // 128
PR = DC // 2
TW = N // 16
EPS = 1e-5
TGS = [(0, 512), (512, 128)]
NT = N // 512
BF = mybir.dt.bfloat16
F32 = mybir.dt.float32
I16 = mybir.dt.int16
U32 = mybir.dt.uint32
OP = mybir.AluOpType
AF = mybir.ActivationFunctionType


def build_program(debug_outputs=False):
    nc = bacc.Bacc("TRN2", target_bir_lowering=False, debug=False)

    xf_d = nc.dram_tensor("xf", [128, DC, N], F32, kind="ExternalInput")
    xb_d = nc.dram_tensor("xb", [N, D], BF, kind="ExternalInput")
    xtb_d = nc.dram_tensor("xtb", [128, DC, N], BF, kind="ExternalInput")
    gw_d = nc.dram_tensor("gw", [128, DC, E], F32, kind="ExternalInput")
    w1_d = nc.dram_tensor("w1", [E, FC, DC, 128, 128], BF, kind="ExternalInput")
    w2_d = nc.dram_tensor("w2", [E, DC, FC, 128, 128], BF, kind="ExternalInput")
    sw1_d = nc.dram_tensor("sw1", [FC, DC, 128, 128], BF, kind="ExternalInput")
    sw2_d = nc.dram_tensor("sw2", [DC, FC, 128, 128], BF, kind="ExternalInput")
    b1_d = nc.dram_tensor("b1t", [128, E, FC], F32, kind="ExternalInput")
    b2_d = nc.dram_tensor("b2t", [128, E, DC], F32, kind="ExternalInput")
    sb1_d = nc.dram_tensor("sb1t", [128, FC], F32, kind="ExternalInput")
    sb2_d = nc.dram_tensor("sb2t", [128, DC], F32, kind="ExternalInput")
    gbt_d = nc.dram_tensor("gbt", [128, PR, 2], F32, kind="ExternalInput")
    bbt_d = nc.dram_tensor("bbt", [128, PR, 2], F32, kind="ExternalInput")
    ones8_d = nc.dram_tensor("ones8", [1, E], F32, kind="ExternalInput")
    ones128_d = nc.dram_tensor("ones128", [1, 128], F32, kind="ExternalInput")
    iota_d = nc.dram_tensor("iota_wf", [16, TW], F32, kind="ExternalInput")
    eCf_d = nc.dram_tensor("eCf", [E, 1], F32, kind="ExternalInput")

    out_d = nc.dram_tensor("outp", [128, PR, N, 2], F32, kind="ExternalOutput")
    dbg = {}
    if debug_outputs:
        dbg["logits"] = nc.dram_tensor("dbg_logits", [E, N], F32, kind="ExternalOutput")
        dbg["acc"] = nc.dram_tensor("dbg_acc", [128, PR, N, 2], F32, kind="ExternalOutput")

    with tile.TileContext(nc) as tc:
        with tc.tile_pool(name="const", bufs=1) as cpool, \
             tc.tile_pool(name="persist", bufs=1) as ppool, \
             tc.tile_pool(name="dramp", bufs=1, space="DRAM") as dpool, \
             tc.tile_pool(name="psA", bufs=1, space="PSUM") as psA, \
             tc.tile_pool(name="psW", bufs=1, space="PSUM") as psW:

            ones8 = cpool.tile([1, E], F32)
            nc.sync.dma_start(ones8[:], ones8_d[:])
            onecol8 = cpool.tile([E, 1], F32)
            nc.sync.dma_start(onecol8[:], ones8_d[0:1, :].rearrange("o (e u) -> (o e) u", u=1))
            ones128 = cpool.tile([1, 128], F32)
            nc.sync.dma_start(ones128[:], ones128_d[:])
            onecol128 = cpool.tile([128, 1], F32)
            nc.sync.dma_start(onecol128[:], ones128_d[0:1, :].rearrange("o (e u) -> (o e) u", u=1))
            iota_wf = cpool.tile([16, TW], F32)
            nc.sync.dma_start(iota_wf[:], iota_d[:])
            eCf = cpool.tile([E, 1], F32)
            nc.sync.dma_start(eCf[:], eCf_d[:])
            b1t = cpool.tile([128, E, FC], F32)
            nc.sync.dma_start(b1t[:], b1_d[:])
            b2t = cpool.tile([128, E, DC], F32)
            nc.sync.dma_start(b2t[:], b2_d[:])
            sb1t = cpool.tile([128, FC], F32)
            nc.sync.dma_start(sb1t[:], sb1_d[:])
            sb2t = cpool.tile([128, DC], F32)
            nc.sync.dma_start(sb2t[:], sb2_d[:])
            gbt = cpool.tile([128, PR, 2], F32)
            nc.sync.dma_start(gbt[:], gbt_d[:])
            bbt = cpool.tile([128, PR, 2], F32)
            nc.sync.dma_start(bbt[:], bbt_d[:])
            gwt = cpool.tile([128, DC, E], F32)
            nc.sync.dma_start(gwt[:], gw_d[:])

            srcw1 = ppool.tile([128, TW], I16)
            srcw2 = ppool.tile([128, TW], I16)
            idxw = ppool.tile([128, E * CW], I16)
            w1r = ppool.tile([1, N], F32)
            w2r = ppool.tile([1, N], F32)
            acc_d = dpool.tile([128, PR, N, 2], F32, name="acc_dt")

            # ---------- phase 1: gate + routing ----------
            with tc.tile_pool(name="rtmp", bufs=1) as rt, \
                 tc.tile_pool(name="gx", bufs=1) as gx:
                L = rt.tile([E, N], F32)
                for ts in range(NT):
                    xfg = gx.tile([128, DC, 512], F32, name=f"xfg{ts}", tag="xfg")
                    nc.sync.dma_start(xfg[:], xf_d[:, :, ts * 512:(ts + 1) * 512])
                    lps = psA.tile([E, 512], F32, name=f"lps{ts}", tag="psas", bufs=2)
                    for dc in range(DC):
                        nc.tensor.matmul(lps[:], gwt[:, dc, :], xfg[:, dc, :],
                                         start=(dc == 0), stop=(dc == DC - 1))
                    nc.vector.tensor_copy(out=L[:, ts * 512:(ts + 1) * 512], in_=lps[:])
                if debug_outputs:
                    nc.sync.dma_start(dbg["logits"][:], L[:])

                def ptree_max(src_t, tagp):
                    cur = src_t
                    n = E
                    lvl = 0
                    while n > 1:
                        half = n // 2
                        lo = rt.tile([half, N], F32, name=f"tl{tagp}{lvl}", tag="tlo", bufs=2)
                        nc.sync.dma_start(lo[:], cur[half:n, :])
                        red = rt.tile([half, N], F32, name=f"tr{tagp}{lvl}", tag=f"tred{lvl}", bufs=1)
                        nc.vector.tensor_tensor(out=red[:], in0=cur[0:half, :], in1=lo[:], op=OP.max)
                        cur = red
                        n = half
                        lvl += 1
                    return cur

                m1 = rt.tile([1, N], F32)
                nc.vector.tensor_copy(out=m1[:], in_=ptree_max(L, "a")[:])
                eq1 = rt.tile([E, N], F32)
                for ts in range(NT):
                    sl = slice(ts * 512, (ts + 1) * 512)
                    mb = psA.tile([E, 512], F32, name=f"m1b{ts}", tag="psas", bufs=2)
                    nc.tensor.matmul(mb[:], ones8[:], m1[0:1, sl], start=True, stop=True)
                    nc.vector.tensor_tensor(out=eq1[:, sl], in0=L[:, sl], in1=mb[:], op=OP.is_equal)
                msk = rt.tile([E, N], F32)
                nc.vector.scalar_tensor_tensor(out=msk[:], in0=eq1[:], scalar=-1e30, in1=L[:],
                                               op0=OP.mult, op1=OP.add)
                m2 = rt.tile([1, N], F32)
                nc.vector.tensor_copy(out=m2[:], in_=ptree_max(msk, "b")[:])
                eq2 = rt.tile([E, N], F32)
                for ts in range(NT):
                    sl = slice(ts * 512, (ts + 1) * 512)
                    mb = psA.tile([E, 512], F32, name=f"m2b{ts}", tag="psas", bufs=2)
                    nc.tensor.matmul(mb[:], ones8[:], m2[0:1, sl], start=True, stop=True)
                    nc.vector.tensor_tensor(out=eq2[:, sl], in0=L[:, sl], in1=mb[:], op=OP.is_equal)

                dlt = rt.tile([1, N], F32)
                nc.vector.tensor_tensor(out=dlt[:], in0=m1[:], in1=m2[:], op=OP.subtract)
                nc.scalar.activation(out=w1r[:], in_=dlt[:], func=AF.Sigmoid)
                nc.vector.tensor_scalar(out=w2r[:], in0=w1r[:], scalar1=-1.0, scalar2=-1.0,
                                        op0=OP.mult, op1=OP.subtract)

                mk = rt.tile([E, N], F32)
                nc.vector.tensor_tensor(out=mk[:], in0=eq1[:], in1=eq2[:], op=OP.add)
                zer = rt.tile([E, N], F32)
                nc.vector.memset(zer[:], 0.0)
                Sinc = rt.tile([E, N], F32)
                nc.vector.tensor_tensor_scan(out=Sinc[:], data0=mk[:], data1=zer[:], initial=0.0,
                                             op0=OP.add, op1=OP.add)
                Sexc = rt.tile([E, N], F32)
                nc.vector.tensor_tensor(out=Sexc[:], in0=Sinc[:], in1=mk[:], op=OP.subtract)

                t0 = rt.tile([E, N], F32)
                for nm, eq, dstw in (("1", eq1, srcw1), ("2", eq2, srcw2)):
                    nc.vector.scalar_tensor_tensor(out=t0[:], in0=Sexc[:], scalar=eCf[:], in1=eq[:],
                                                   op0=OP.add, op1=OP.mult)
                    row = rt.tile([1, N], F32, name=f"srow{nm}", tag="srow", bufs=2)
                    for ts in range(NT):
                        sl = slice(ts * 512, (ts + 1) * 512)
                        sps = psA.tile([1, 512], F32, name=f"sps{nm}{ts}", tag="psas", bufs=2)
                        nc.tensor.matmul(sps[:], onecol8[:], t0[:, sl], start=True, stop=True)
                        nc.vector.tensor_copy(out=row[:, sl], in_=sps[:])
                    s16 = rt.tile([1, N], I16, name=f"s16{nm}", tag="s16", bufs=2)
                    nc.vector.tensor_copy(out=s16[:], in_=row[:])
                    srcb = dpool.tile([1, N], I16, name=f"srcb{nm}")
                    nc.sync.dma_start(srcb[:], s16[:])
                    for k in range(8):
                        nc.sync.dma_start(dstw[16 * k:16 * k + 16, :],
                                          srcb[0:1, :].rearrange("o (c j) -> (o j) c", j=16))

                mkb = dpool.tile([E, N], F32, name="mkb")
                nc.sync.dma_start(mkb[:], mk[:])
                mk_wall = rt.tile([16, E * TW], F32)
                nc.sync.dma_start(mk_wall[:], mkb[:].rearrange("e (c j) -> j (e c)", j=16))
                idx_all = rt.tile([16, E * CW], F32)
                for e in range(E):
                    val = rt.tile([16, TW], F32, name=f"val{e}", tag="val", bufs=2)
                    nc.vector.scalar_tensor_tensor(out=val[:], in0=iota_wf[:], scalar=1.0,
                                                   in1=mk_wall[:, e * TW:(e + 1) * TW],
                                                   op0=OP.add, op1=OP.mult)
                    nc.vector.tensor_scalar(out=val[:], in0=val[:], scalar1=1.0, scalar2=None,
                                            op0=OP.subtract)
                    nf = rt.tile([1, 1], U32, name=f"nf{e}", tag="nf", bufs=2)
                    nc.gpsimd.sparse_gather(idx_all[:, e * CW:(e + 1) * CW], val[:], num_found=nf[:])
                nc.vector.tensor_scalar(out=idx_all[:], in0=idx_all[:], scalar1=0.0,
                                        scalar2=float(N - 1), op0=OP.max, op1=OP.min)
                idx16 = rt.tile([16, E * CW], I16)
                nc.vector.tensor_copy(out=idx16[:], in_=idx_all[:])
                idxb = dpool.tile([16, E * CW], I16, name="idxb")
                nc.sync.dma_start(idxb[:], idx16[:])
                for k in range(8):
                    nc.sync.dma_start(idxw[16 * k:16 * k + 16, :], idxb[:])

            with tc.tile_pool(name="ycat", bufs=1) as ypool:
                ycat = [ypool.tile([128, NE, 2], BF, name=f"ycat{pr}") for pr in range(PR)]
                # ---------- phase 2: dispatch gathers + shared expert + residual ----------
                with tc.tile_pool(name="xgp", bufs=2) as xgp:
                    xgs = []
                    for e in range(E):
                        xg = xgp.tile([128, DC, C], BF, name=f"xg{e}", tag="xg")
                        nc.gpsimd.dma_gather(xg[:], xb_d[:], idxw[:, e * CW:(e + 1) * CW],
                                             num_idxs=C, num_idxs_reg=C, elem_size=D, transpose=True)
                        xgs.append(xg)

                    with tc.tile_pool(name="shx", bufs=2) as shx, \
                         tc.tile_pool(name="shh", bufs=2) as shh, \
                         tc.tile_pool(name="sblk", bufs=2) as sblk, \
                         tc.tile_pool(name="stage", bufs=3) as stpool, \
                         tc.tile_pool(name="xrp", bufs=3) as xrp:
                        for tg in range(NT):
                            tsl = slice(tg * 512, (tg + 1) * 512)
                            xtg = shx.tile([128, DC, 512], BF, name=f"xtg{tg}", tag="xtg")
                            nc.sync.dma_start(xtg[:], xtb_d[:, :, tsl])
                            htg = shh.tile([128, FC, 512], BF, name=f"htg{tg}", tag="htg")
                            for fc in range(FC):
                                blk = sblk.tile([128, DC, 128], BF, name=f"sw1b{tg}_{fc}", tag="sw1b")
                                nc.sync.dma_start(blk[:], sw1_d[fc].rearrange("dc p f -> p dc f"))
                                hps = psW.tile([128, 512], F32, name=f"shps{fc}{tg}", tag="hps", bufs=2)
                                for dc in range(DC):
                                    nc.tensor.matmul(hps[:], blk[:, dc, :], xtg[:, dc, :],
                                                     start=(dc == 0), stop=(dc == DC - 1))
                                nc.scalar.activation(out=htg[:, fc, :], in_=hps[:],
                                                     func=AF.Gelu, bias=sb1t[:, fc:fc + 1], scale=1.0)
                            for pr in range(PR):
                                stg = stpool.tile([128, 512, 2], F32, name=f"stg{pr}{tg}", tag="stg")
                                for i in range(2):
                                    dt_ = 2 * pr + i
                                    blk2 = sblk.tile([128, FC, 128], BF, name=f"sw2b{tg}_{dt_}", tag="sw2b")
                                    nc.sync.dma_start(blk2[:], sw2_d[dt_].rearrange("fc p d -> p fc d"))
                                    yps = psW.tile([128, 512], F32, name=f"syps{dt_}{tg}", tag="yps", bufs=2)
                                    for fc in range(FC):
                                        nc.tensor.matmul(yps[:], blk2[:, fc, :], htg[:, fc, :],
                                                         start=(fc == 0), stop=(fc == FC - 1))
                                    xr = xrp.tile([128, 512], F32, name=f"xr{dt_}{tg}", tag="xr")
                                    nc.sync.dma_start(xr[:], xf_d[:, dt_, tsl])
                                    nc.vector.scalar_tensor_tensor(
                                        out=stg[:, :, i], in0=yps[:], scalar=sb2t[:, dt_:dt_ + 1],
                                        in1=xr[:], op0=OP.add, op1=OP.add)
                                nc.sync.dma_start(acc_d[:, pr, tsl, :], stg[:])

                    # ---------- phase 3: expert FFNs ----------
                    with tc.tile_pool(name="exw", bufs=3) as ewpool, \
                         tc.tile_pool(name="exh", bufs=2) as ehpool:
                        for e in (range(E) if _PH >= 3 else []):
                            xg = xgs[e]
                            hT = ehpool.tile([128, FC, C], BF, name=f"hT{e}", tag="hT")
                            for fc in range(FC):
                                blk = ewpool.tile([128, DC, 128], BF, name=f"w1b{e}_{fc}", tag="w1b")
                                nc.sync.dma_start(blk[:], w1_d[e, fc].rearrange("dc p f -> p dc f"))
                                for (t0_, tn) in TGS:
                                    hps = psW.tile([128, tn], F32, name=f"ehps{e}{fc}{t0_}", tag="hps",
                                                   bufs=2, padded_shape=[128, 512])
                                    for dc in range(DC):
                                        nc.tensor.matmul(hps[:], blk[:, dc, :], xg[:, dc, t0_:t0_ + tn],
                                                         start=(dc == 0), stop=(dc == DC - 1))
                                    nc.scalar.activation(out=hT[:, fc, t0_:t0_ + tn], in_=hps[:],
                                                         func=AF.Gelu, bias=b1t[:, e, fc:fc + 1], scale=1.0)
                            for dt_ in range(DC):
                                blk2 = ewpool.tile([128, FC, 128], BF, name=f"w2b{e}_{dt_}", tag="w2b")
                                nc.sync.dma_start(blk2[:], w2_d[e, dt_].rearrange("fc p d -> p fc d"))
                                for (t0_, tn) in TGS:
                                    yps = psW.tile([128, tn], F32, name=f"eyps{e}{dt_}{t0_}", tag="yps",
                                                   bufs=2, padded_shape=[128, 512])
                                    for fc in range(FC):
                                        nc.tensor.matmul(yps[:], blk2[:, fc, :], hT[:, fc, t0_:t0_ + tn],
                                                         start=(fc == 0), stop=(fc == FC - 1))
                                    nc.scalar.activation(
                                        out=ycat[dt_ // 2][:, e * C + t0_:e * C + t0_ + tn, dt_ % 2],
                                        in_=yps[:], func=AF.Identity,
                                        bias=b2t[:, e, dt_:dt_ + 1], scale=1.0)


                    # ---------- phase 4: combine + LN stats ----------
                with tc.tile_pool(name="comb", bufs=1) as cb, \
                     tc.tile_pool(name="cacc", bufs=1) as ca:
                    w1bc = cb.tile([128, N], F32, name="w1bc", tag="bcbig", bufs=2)
                    w2bc = cb.tile([128, N], F32, name="w2bc", tag="bcbig", bufs=2)
                    for wr, wbc in ((w1r, w1bc), (w2r, w2bc)):
                        for g in range(N // 512):
                            sl = slice(g * 512, (g + 1) * 512)
                            bps = psA.tile([128, 512], F32, name=f"wb{g}", tag="wbc", bufs=2)
                            nc.tensor.matmul(bps[:], ones128[:], wr[0:1, sl], start=True, stop=True)
                            nc.vector.tensor_copy(out=wbc[:, sl], in_=bps[:])

                    statS = cb.tile([1, N], F32)
                    nc.vector.memset(statS[:], 0.0)
                    statQ = cb.tile([1, N], F32)
                    nc.vector.memset(statQ[:], 0.0)
                    for pr in (range(PR) if _PH >= 4 else []):
                        acc_t = ca.tile([128, N, 2], F32, name=f"acc{pr}", tag="acc", bufs=1)
                        nc.sync.dma_start(acc_t[:], acc_d[:, pr])
                        g1 = ca.tile([128, N, 2], BF, name=f"g1_{pr}", tag="g1")
                        nc.gpsimd.ap_gather(g1[:], ycat[pr][:], srcw1[:], channels=128,
                                            num_elems=NE, d=2, num_idxs=N)
                        g2 = ca.tile([128, N, 2], BF, name=f"g2_{pr}", tag="g2")
                        nc.gpsimd.ap_gather(g2[:], ycat[pr][:], srcw2[:], channels=128,
                                            num_elems=NE, d=2, num_idxs=N)
                        tmp = ca.tile([128, N], F32, name=f"tmp{pr}", tag="tmp", bufs=1)
                        for i in range(2):
                            nc.vector.tensor_tensor(out=tmp[:], in0=g1[:, :, i], in1=w1bc[:], op=OP.mult)
                            nc.vector.tensor_tensor(out=acc_t[:, :, i], in0=acc_t[:, :, i],
                                                    in1=tmp[:], op=OP.add)
                            nc.vector.tensor_tensor(out=tmp[:], in0=g2[:, :, i], in1=w2bc[:], op=OP.mult)
                            nc.vector.tensor_tensor(out=acc_t[:, :, i], in0=acc_t[:, :, i],
                                                    in1=tmp[:], op=OP.add)
                        for i in range(2):
                            for g in range(N // 512):
                                sl = slice(g * 512, (g + 1) * 512)
                                sps = psA.tile([1, 512], F32, name=f"stS{pr}{i}{g}", tag="psas", bufs=2)
                                nc.tensor.matmul(sps[:], onecol128[:], acc_t[:, sl, i],
                                                 start=True, stop=True)
                                nc.vector.tensor_tensor(out=statS[0:1, sl], in0=statS[0:1, sl],
                                                        in1=sps[:], op=OP.add)
                            nc.vector.tensor_tensor(out=tmp[:], in0=acc_t[:, :, i],
                                                    in1=acc_t[:, :, i], op=OP.mult)
                            for g in range(N // 512):
                                sl = slice(g * 512, (g + 1) * 512)
                                qps = psA.tile([1, 512], F32, name=f"stQ{pr}{i}{g}", tag="psas", bufs=2)
                                nc.tensor.matmul(qps[:], onecol128[:], tmp[:, sl],
                                                 start=True, stop=True)
                                nc.vector.tensor_tensor(out=statQ[0:1, sl], in0=statQ[0:1, sl],
                                                        in1=qps[:], op=OP.add)
                        nc.sync.dma_start(acc_d[:, pr], acc_t[:])
                        if debug_outputs:
                            nc.sync.dma_start(dbg["acc"][:, pr], acc_t[:])

                    # ---------- phase 5: LN finalize ----------
                    mu = cb.tile([1, N], F32, name="mu", tag="lnrow", bufs=4)
                    nc.vector.tensor_scalar(out=mu[:], in0=statS[:], scalar1=1.0 / D,
                                            scalar2=None, op0=OP.mult)
                    var = cb.tile([1, N], F32, name="var", tag="lnrow", bufs=4)
                    nc.vector.tensor_tensor(out=var[:], in0=mu[:], in1=mu[:], op=OP.mult)
                    nc.vector.scalar_tensor_tensor(out=var[:], in0=statQ[:], scalar=1.0 / D,
                                                   in1=var[:], op0=OP.mult, op1=OP.subtract)
                    nc.vector.tensor_scalar(out=var[:], in0=var[:], scalar1=EPS, scalar2=None,
                                            op0=OP.add)
                    sd = cb.tile([1, N], F32, name="sd", tag="lnrow", bufs=4)
                    nc.scalar.activation(out=sd[:], in_=var[:], func=AF.Sqrt, bias=0.0, scale=1.0)
                    rstd = cb.tile([1, N], F32, name="rstd", tag="lnrow", bufs=4)
                    nc.vector.reciprocal(out=rstd[:], in_=sd[:])
                    mubc = cb.tile([128, N], F32, name="mubc", tag="bcbig", bufs=2)
                    rstdbc = cb.tile([128, N], F32, name="rstdbc", tag="bcbig", bufs=2)
                    for r, bc in ((mu, mubc), (rstd, rstdbc)):
                        for g in range(N // 512):
                            sl = slice(g * 512, (g + 1) * 512)
                            bps = psA.tile([128, 512], F32, name=f"nb{g}", tag="wbc", bufs=2)
                            nc.tensor.matmul(bps[:], ones128[:], r[0:1, sl], start=True, stop=True)
                            nc.vector.tensor_copy(out=bc[:, sl], in_=bps[:])
                    for pr in range(PR):
                        acc_t = ca.tile([128, N, 2], F32, name=f"acf{pr}", tag="acc", bufs=1)
                        nc.sync.dma_start(acc_t[:], acc_d[:, pr])
                        for i in range(2):
                            nc.vector.tensor_tensor(out=acc_t[:, :, i], in0=acc_t[:, :, i],
                                                    in1=mubc[:], op=OP.subtract)
                            nc.vector.tensor_tensor(out=acc_t[:, :, i], in0=acc_t[:, :, i],
                                                    in1=rstdbc[:], op=OP.mult)
                            nc.vector.tensor_scalar(out=acc_t[:, :, i], in0=acc_t[:, :, i],
                                                    scalar1=gbt[:, pr, i:i + 1],
                                                    scalar2=bbt[:, pr, i:i + 1],
                                                    op0=OP.mult, op1=OP.add)
                        nc.sync.dma_start(out_d[:, pr], acc_t[:])
    nc.compile()
    return nc


# ---------------- host side ----------------

def _shared_consts(gate_w, W1, b1, W2, b2, sW1, sb1, sW2, sb2, gamma, beta):
    c = {}
    c["gw"] = np.ascontiguousarray(gate_w.reshape(DC, 128, E).transpose(1, 0, 2), dtype=np.float32)
    c["w1"] = np.ascontiguousarray(
        W1.reshape(E, DC, 128, FC, 128).transpose(0, 3, 1, 2, 4).astype(ml_dtypes.bfloat16))
    c["w2"] = np.ascontiguousarray(
        W2.reshape(E, FC, 128, DC, 128).transpose(0, 3, 1, 2, 4).astype(ml_dtypes.bfloat16))
    c["sw1"] = np.ascontiguousarray(
        sW1.reshape(DC, 128, FC, 128).transpose(2, 0, 1, 3).astype(ml_dtypes.bfloat16))
    c["sw2"] = np.ascontiguousarray(
        sW2.reshape(FC, 128, DC, 128).transpose(2, 0, 1, 3).astype(ml_dtypes.bfloat16))
    c["b1t"] = np.ascontiguousarray(b1.reshape(E, FC, 128).transpose(2, 0, 1), dtype=np.float32)
    c["b2t"] = np.ascontiguousarray(b2.reshape(E, DC, 128).transpose(2, 0, 1), dtype=np.float32)
    c["sb1t"] = np.ascontiguousarray(sb1.reshape(FC, 128).T, dtype=np.float32)
    c["sb2t"] = np.ascontiguousarray(sb2.reshape(DC, 128).T, dtype=np.float32)
    c["gbt"] = np.ascontiguousarray(gamma.reshape(PR, 2, 128).transpose(2, 0, 1), dtype=np.float32)
    c["bbt"] = np.ascontiguousarray(beta.reshape(PR, 2, 128).transpose(2, 0, 1), dtype=np.float32)
    c["ones8"] = np.ones((1, E), np.float32)
    c["ones128"] = np.ones((1, 128), np.float32)
    c["iota_wf"] = np.arange(N, dtype=np.float32).reshape(TW, 16).T.copy()
    c["eCf"] = (np.arange(E, dtype=np.float32) * C)[:, None].copy()
    return c


def _core_inputs(xc, consts):
    m = dict(consts)
    xT = xc.reshape(N, DC, 128).transpose(2, 1, 0)
    m["xf"] = np.ascontiguousarray(xT, dtype=np.float32)
    m["xtb"] = np.ascontiguousarray(xT.astype(ml_dtypes.bfloat16))
    m["xb"] = np.ascontiguousarray(xc.astype(ml_dtypes.bfloat16))
    return m


_prog_cache = {}


def _get_program(debug_outputs=False):
    key = bool(debug_outputs)
    if key not in _prog_cache:
        _prog_cache[key] = build_program(debug_outputs=key)
    return _prog_cache[key]


def run_on_cores(x, consts, debug_outputs=False, trace=False):
    from concourse.bass_utils import run_bass_kernel_spmd
    in_maps = [_core_inputs(x[ci * N:(ci + 1) * N], consts) for ci in range(NCORE)]
    prog = _get_program(debug_outputs)
    return run_bass_kernel_spmd(prog, in_maps, list(range(NCORE)), trace=trace)


def kernel(x, gate_w, W1, b1, W2, b2, sW1, sb1, sW2, sb2, gamma, beta):
    x = np.asarray(x, dtype=np.float32)
    consts = _shared_consts(np.asarray(gate_w, np.float32), np.asarray(W1, np.float32),
                            np.asarray(b1, np.float32), np.asarray(W2, np.float32),
                            np.asarray(b2, np.float32), np.asarray(sW1, np.float32),
                            np.asarray(sb1, np.float32), np.asarray(sW2, np.float32),
                            np.asarray(sb2, np.float32), np.asarray(gamma, np.float32),
                            np.asarray(beta, np.float32))
    res = run_on_cores(x, consts)
    out = np.empty((B, D), np.float32)
    for ci in range(NCORE):
        r = res.results[ci]["outp"]
        out[ci * N:(ci + 1) * N] = r.transpose(2, 1, 3, 0).reshape(N, D)
    return out

